# revision 37
# baseline (speedup 1.0000x reference)
"""Trainium2 Bass kernel for nn_CNN_Nested (W2NER-style CNN scorer).

Math (reference):
  head = leaky(wr @ head_w.T + head_b); tail likewise           [B,N,D]
  scores1[b,(h,d),l,k] = sum_{x,y} head[b,l,h,x] U[h,d,x,y] tail[b,k,h,y]
  scores2[b,c,m,n] = h_aug@Wh.T (bcast n) + t_aug@Wt.T (bcast m) + size-emb
  out = down_w @ (scores1+scores2) + down_b                     [B,OUT,N,N]

down_fc is linear => fold down_w into the constants on the host:
  U'[o,h,x,y] = sum_d down_w[o,h*HD+d] U[h,d,x,y]
  WhD = down_w @ Wh, WtD = down_w @ Wt               (tiny)
  E[o,m,n] = (size_emb @ (down_w@Ws).T)[clip(n-m)+15, o] (+ consts)

The device computes ONLY the biaffine part:
  G[o] = blockdiag(U'[o])^T @ tailT                  [(h,x)=200, N]
  dev[o] = headT^T @ G[o]                            [N, N]  (bf16 out)
headT/tailT (the leaky MLP outputs) are returned as a tiny extra
output; the HOST computes the rank-1 broadcasts A' = WhD@headT,
B' = WtD@tailT, the Toeplitz size-embed term E, down_b and the
ones-column constants, all in f32:
  out = dev + A'[:, :, None] + B'[:, None, :] + E.

Hardware notes baked into the schedule (from NTFF traces):
- DMA: ~18ns/descriptor, one per partition row; rows aggregate into
  large bursts ONLY when the DRAM side is a whole transfer-shaped
  tensor, so every transfer gets its own DRAM tensor. gpsimd's
  software DGE fans packets across all 16 DMA engines in parallel and
  carries the bulk; sync/scalar (HWDGE) take the latency-critical
  first chunks.
- PSUM: 8 banks; a static bank plan (A/B group psums sharing banks at
  disjoint columns) avoids pool recycling, whose write-after-read
  hazards would serialize the g stage behind all four MLP evictions.
- PE: consecutive accumulation into one PSUM bank halves issue rate,
  so the out stage interleaves two banks.

Sharding: 8 cores = B(4) x o-half(2x6). No collectives. Full inputs in,
full output out. Hardcoded B=4,N=256,H=768,D=200,NH=5,HD=40,OUT=12.
"""

import os
import numpy as np

B, N, H = 4, 256, 768
D, NH, HD, SZ, OUT = 200, 5, 40, 25, 12
N_POS = 30
OH = OUT // 2          # o's per core
NCORES = 8
GA, GB = 3 * HD, 2 * HD  # 120 / 80: d-rows in partition group A / B
CH = N + 2 * D           # one blob chunk: [wrt_k | tw_k | hw_k] = 656
MLPC = 6 * CH            # 3936
CBDA = MLPC              # bda image cols [120 rows used]
CPW = CBDA + OH * GA
CBDB = CPW + 4 * OH      # bdb image cols [80 rows used]
TOTC = CBDB + OH * GB    # 5160

_cache = {}
LAST_RESULT = None


def _build_module(has_bias: bool):
    import concourse.bacc as bacc
    import concourse.mybir as mybir
    import concourse.tile as tile
    from concourse.bass import ts
    from contextlib import ExitStack

    dt = mybir.dt
    f32 = dt.float32
    bf = dt.bfloat16
    LRELU = mybir.ActivationFunctionType.Lrelu

    nc = bacc.Bacc("TRN2", target_bir_lowering=False, debug=False,
                   enable_asserts=False, enable_partition_id=False)

    c0_d = nc.dram_tensor("c0", [128, CH], bf, kind="ExternalInput").ap()
    c1_d = nc.dram_tensor("c1", [128, CH], bf, kind="ExternalInput").ap()
    c2_d = nc.dram_tensor("c2", [128, CH], bf, kind="ExternalInput").ap()
    c3_d = nc.dram_tensor("c3", [128, CH], bf, kind="ExternalInput").ap()
    c45_d = nc.dram_tensor("c45", [128, 2 * CH], bf,
                           kind="ExternalInput").ap()
    cc_d = nc.dram_tensor("cc", [128, TOTC - MLPC], bf,
                          kind="ExternalInput").ap()
    if has_bias:
        bias_d = nc.dram_tensor("bias", [GA, 4], f32, kind="ExternalInput").ap()
    out_d = nc.dram_tensor("out", [3, 2, 128, 512], bf,
                           kind="ExternalOutput").ap()
    # the very last 512-col half ships as two quarter blocks on two
    # queues so its cast+store pipeline is half as deep
    o2q_d = nc.dram_tensor("o2q", [2, 128, 256], bf,
                           kind="ExternalOutput").ap()
    ht_d = nc.dram_tensor("ht", [128, 4 * N], bf, kind="ExternalOutput").ap()

    with tile.TileContext(nc) as tc, ExitStack() as ctx:
        sb = ctx.enter_context(tc.tile_pool(name="sb", bufs=1))
        # static PSUM plan: 4 MLP banks + 2 g banks + 2 out banks = 8.
        # (Interleaved accumulation chains must NOT share a bank even at
        # disjoint columns -- the accumulate read-modify-write races.)
        pp = ctx.enter_context(tc.tile_pool(name="pp", bufs=1, space="PSUM"))
        pg = ctx.enter_context(tc.tile_pool(name="pg", bufs=2, space="PSUM"))
        po = ctx.enter_context(tc.tile_pool(name="po", bufs=2, space="PSUM"))

        c0_s = sb.tile([128, CH], bf, tag="c0", name="c0")
        nc.sync.dma_start(c0_s[:], c0_d[:, :])
        c1_s = sb.tile([128, CH], bf, tag="c1", name="c1")
        nc.gpsimd.dma_start(c1_s[:], c1_d[:, :])
        c2_s = sb.tile([128, CH], bf, tag="c2", name="c2")
        nc.scalar.dma_start(c2_s[:], c2_d[:, :])
        c3_s = sb.tile([128, CH], bf, tag="c3", name="c3")
        nc.sync.dma_start(c3_s[:], c3_d[:, :])
        c45_s = sb.tile([128, 2 * CH], bf, tag="c45", name="c45")
        nc.gpsimd.dma_start(c45_s[:], c45_d[:, :])
        cc_s = sb.tile([128, TOTC - MLPC], bf, tag="cc", name="cc")
        nc.gpsimd.dma_start(cc_s[:], cc_d[:, :])
        if has_bias:
            bias_s = sb.tile([GA, 4], f32, tag="bias", name="bias")
            nc.scalar.dma_start(bias_s[:], bias_d[:, :])

        def _seg(k):
            if k == 0:
                return c0_s, 0
            if k == 1:
                return c1_s, 0
            if k == 2:
                return c2_s, 0
            if k == 3:
                return c3_s, 0
            return c45_s, (k - 4) * CH

        def wrT(k):
            t, c = _seg(k)
            return t[:, c:c + N]

        def tw_slice(k, off, sz):
            t, c = _seg(k)
            return t[:, c + N + off:c + N + off + sz]

        def hw_slice(k, off, sz):
            t, c = _seg(k)
            return t[:, c + N + D + off:c + N + D + off + sz]

        def bda_sl(j):
            return cc_s[0:GA, j * GA:(j + 1) * GA]

        def bdb_sl(j):
            c0 = CBDB - MLPC
            return cc_s[0:GB, c0 + j * GB:c0 + (j + 1) * GB]

        # ---- headT/tailT = leaky(w @ wr^T + b), [d, l] layout ---------------
        # Chunk-major over the H contraction; all four leaky outputs
        # land in ONE [128,1024] tile so a single aggregated DMA ships
        # them to the host for the A'/B' projections.
        ht_s = sb.tile([128, 4 * N], bf, tag="ht", name="ht")
        tailT_A = ht_s[0:GA, 0 * N:1 * N]
        tailT_B = ht_s[0:GB, 1 * N:2 * N]
        headT_A = ht_s[0:GA, 2 * N:3 * N]
        headT_B = ht_s[0:GB, 3 * N:4 * N]

        pm = {t: pp.tile([sz, N], f32, tag=f"pm{t}", name=f"pm{t}", bufs=1)
              for t, sz in (("tA", GA), ("tB", GB), ("hA", GA), ("hB", GB))}
        groups = [
            ("tA", tw_slice, 0, GA, pm["tA"][:], tailT_A, 2),
            ("tB", tw_slice, GA, GB, pm["tB"][:], tailT_B, 3),
            ("hA", hw_slice, 0, GA, pm["hA"][:], headT_A, 0),
            ("hB", hw_slice, GA, GB, pm["hB"][:], headT_B, 1),
        ]
        for k in range(6):
            for tag, wsl, off, sz, ps, _, _ in groups:
                nc.tensor.matmul(ps, wsl(k, off, sz), wrT(k),
                                 start=(k == 0), stop=(k == 5))

        for tag, _, off, sz, ps, dst, bc in groups:
            if tag == "tB" and not has_bias:
                # run the second tail eviction on the vector engine so
                # the g stage isn't gated on scalar's serial queue
                tmp = sb.tile([GB, N], f32, tag="ltmp", name="ltmp")
                nc.vector.tensor_scalar_mul(tmp[:], ps, 0.01)
                nc.vector.tensor_max(dst, ps, tmp[:])
                continue
            bias = bias_s[0:sz, bc:bc + 1] if has_bias else 0.0
            nc.scalar.activation(dst, ps, LRELU, bias=bias, alpha=0.01)
        nc.sync.dma_start(ht_d[:, :], ht_s[:])

        gAt, gBt = [], []

        def g_build(p):
            gA = sb.tile([GA, 512], bf, tag=f"gA{p}", name=f"gA{p}")
            gB = sb.tile([GB, 512], bf, tag=f"gB{p}", name=f"gB{p}")
            for half in range(2):
                j = 2 * p + half
                # one PSUM bank holds both group psums at disjoint cols
                psg = pg.tile([GA, 512], f32, tag="psg", name="psg")
                nc.tensor.matmul(psg[:, 0:N], bda_sl(j),
                                 tailT_A, start=True, stop=True)
                nc.scalar.copy(gA[:, ts(half, N)], psg[:, 0:N])
                nc.tensor.matmul(psg[0:GB, N:2 * N], bdb_sl(j),
                                 tailT_B, start=True, stop=True)
                nc.vector.tensor_copy(gB[:, ts(half, N)], psg[0:GB, N:2 * N])
            gAt.append(gA)
            gBt.append(gB)

        def out_bank(p):
            out_s = sb.tile([128, 1024], bf, tag=f"os{p}", name=f"os{p}")
            obs = [po.tile([128, 512], f32, tag="ob", name="ob")
                   for _ in range(2)]
            # interleave the two PSUM banks: consecutive accumulation
            # into one bank stalls the PE at half rate. The last bank
            # finishes lt=0 completely first so its cast+store overlap
            # the lt=1 matmuls.
            if p < 2:
                for lt in range(2):
                    nc.tensor.matmul(obs[lt][:], headT_A[:, ts(lt, 128)],
                                     gAt[p][:], start=True, stop=False)
                for lt in range(2):
                    nc.tensor.matmul(obs[lt][:], headT_B[:, ts(lt, 128)],
                                     gBt[p][:], start=False, stop=True)
            else:
                for lt in range(2):
                    nc.tensor.matmul(obs[lt][:], headT_A[:, ts(lt, 128)],
                                     gAt[p][:], start=True, stop=False)
                    nc.tensor.matmul(obs[lt][:], headT_B[:, ts(lt, 128)],
                                     gBt[p][:], start=False, stop=True)
            # each 512-col half casts and stores independently, into its
            # own contiguous DRAM block, as soon as its bank stops
            nc.vector.tensor_copy(out_s[:, 0:512], obs[0][:])
            nc.gpsimd.dma_start(out_d[p, 0], out_s[:, 0:512])
            if p < 2:
                nc.scalar.copy(out_s[:, 512:1024], obs[1][:])
                nc.gpsimd.dma_start(out_d[p, 1], out_s[:, 512:1024])
            else:
                nc.vector.tensor_copy(out_s[:, 512:768], obs[1][:, 0:256])
                nc.sync.dma_start(o2q_d[0], out_s[:, 512:768])
                nc.scalar.copy(out_s[:, 768:1024], obs[1][:, 256:512])
                nc.scalar.dma_start(o2q_d[1], out_s[:, 768:1024])

        g_build(0)
        g_build(1)
        out_bank(0)
        g_build(2)
        out_bank(1)
        out_bank(2)

    nc.compile()
    return nc


def _get_module(has_bias: bool):
    key = ("mod", has_bias)
    if key not in _cache:
        _cache[key] = _build_module(has_bias)
    return _cache[key]


def _host_pack(head_w, head_b, tail_w, tail_b, U_mh, size_emb, W, down_w,
               down_b):
    """Fold down_w into the constants; build bf16 input blobs + host E."""
    from ml_dtypes import bfloat16
    f64 = np.float64
    d1 = D + 1
    Wh, Wt, Ws = W[:, :d1], W[:, d1:2 * d1], W[:, 2 * d1:]
    WhD = (down_w.astype(f64) @ Wh.astype(f64)).astype(np.float32)   # [OUT,D+1]
    WtD = (down_w.astype(f64) @ Wt.astype(f64)).astype(np.float32)
    WsD = (down_w.astype(f64) @ Ws.astype(f64)).astype(np.float32)   # [OUT,SZ]
    ct = (size_emb.astype(f64) @ WsD.T.astype(f64)).astype(np.float32)
    dw_r = down_w.reshape(OUT, NH, HD)
    Up = np.einsum('ohd,hdxy->ohxy', dw_r.astype(f64),
                   U_mh.astype(f64)).astype(np.float32)              # [OUT,NH,HD,HD]

    idx = np.arange(N)
    span = np.clip(idx[None, :] - idx[:, None], -N_POS // 2,
                   N_POS // 2 - 1) + N_POS // 2
    # E folds: size-embed term, down_fc bias, both ones-column constants.
    E = (ct[span].transpose(2, 0, 1)
         + (down_b + WhD[:, D] + WtD[:, D])[:, None, None])          # [OUT,N,N]

    has_bias = bool(np.any(head_b) or np.any(tail_b))

    def pack_w(wmat):  # [D,H] -> [128, 6*200]
        return np.ascontiguousarray(
            wmat.T.reshape(6, 128, D).transpose(1, 0, 2).reshape(128, 6 * D))

    hwp = pack_w(head_w)
    twp = pack_w(tail_w)
    blob0 = np.zeros((128, TOTC), np.float32)
    for k in range(6):
        blob0[:, k * CH + N:k * CH + N + D] = twp[:, k * D:(k + 1) * D]
        blob0[:, k * CH + N + D:(k + 1) * CH] = hwp[:, k * D:(k + 1) * D]

    blobs_oh = []
    bias_m = None
    for oh in range(2):
        osl = slice(oh * OH, (oh + 1) * OH)
        UpS = Up[osl]
        blob = blob0.copy()
        for h in range(3):
            for o in range(OH):
                blob[h * HD:(h + 1) * HD,
                     CBDA + o * GA + h * HD:CBDA + o * GA + (h + 1) * HD] = \
                    UpS[o, h].T
        for h in range(2):
            for o in range(OH):
                blob[h * HD:(h + 1) * HD,
                     CBDB + o * GB + h * HD:CBDB + o * GB + (h + 1) * HD] = \
                    UpS[o, 3 + h].T
        blobs_oh.append(blob.astype(bfloat16))
    if has_bias:
        bias_m = np.zeros((GA, 4), np.float32)
        bias_m[:, 0] = head_b[0:GA]
        bias_m[0:GB, 1] = head_b[GA:D]
        bias_m[:, 2] = tail_b[0:GA]
        bias_m[0:GB, 3] = tail_b[GA:D]
    return blobs_oh, bias_m, WhD, WtD, E, has_bias


def _ensure_axon():
    """If a host-side jax.config pinned the cpu platform (e.g. to run the
    reference), switch back to the axon/neuron backend for the device run."""
    import jax
    try:
        if any(getattr(d, 'platform', '') == 'axon' for d in jax.devices()):
            return
    except Exception:
        pass
    try:
        import jax.extend
        jax.config.update('jax_platforms', 'axon')
        jax.extend.backend.clear_backends()
    except Exception:
        pass


def kernel(word_reps, cls_embeding=None, pieces_index=None, loss_mask=None,
           head_w=None, head_b=None, tail_w=None, tail_b=None, U_mh=None,
           size_emb=None, W=None, down_w=None, down_b=None, **_unused):
    global LAST_RESULT
    from concourse import bass_utils
    from ml_dtypes import bfloat16

    word_reps = np.asarray(word_reps, np.float32)
    args = [np.asarray(a, np.float32) for a in
            (head_w, head_b, tail_w, tail_b, U_mh, size_emb, W, down_w,
             down_b)]
    blobs_oh, bias_m, WhD, WtD, E, has_bias = _host_pack(*args)

    nc = _get_module(has_bias)

    wrt_b = []
    for b in range(B):
        wrt = word_reps[b].T.reshape(6, 128, N).transpose(1, 0, 2) \
            .reshape(128, 6 * N)
        wrt_b.append(wrt.astype(bfloat16))
    in_maps = []
    for core in range(NCORES):
        b, oh = core // 2, core % 2
        blob = blobs_oh[oh].copy()
        for k in range(6):
            blob[:, k * CH:k * CH + N] = wrt_b[b][:, k * N:(k + 1) * N]
        m = dict(c0=np.ascontiguousarray(blob[:, 0:CH]),
                 c1=np.ascontiguousarray(blob[:, CH:2 * CH]),
                 c2=np.ascontiguousarray(blob[:, 2 * CH:3 * CH]),
                 c3=np.ascontiguousarray(blob[:, 3 * CH:4 * CH]),
                 c45=np.ascontiguousarray(blob[:, 4 * CH:6 * CH]),
                 cc=np.ascontiguousarray(blob[:, MLPC:TOTC]))
        if has_bias:
            m['bias'] = bias_m
        in_maps.append(m)

    _ensure_axon()

    trace = bool(os.environ.get("KERNEL_TRACE"))
    res = bass_utils.run_bass_kernel_spmd(nc, in_maps, list(range(NCORES)),
                                          trace=trace)
    LAST_RESULT = res

    out = np.empty((B, OUT, N, N), np.float32)
    for core in range(NCORES):
        b, oh = core // 2, core % 2
        osl = slice(oh * OH, (oh + 1) * OH)
        # out_d layout: [p, t, q, (o2, n)] with o = 2p+o2, m = t*128+q
        raw = res.results[core]["out"].copy()
        o2q = res.results[core]["o2q"]                 # [2,128,256] p2-lt1
        raw[2, 1] = np.concatenate([o2q[0], o2q[1]], axis=1)
        dev = raw.astype(np.float32) \
            .reshape(3, 2, 128, 2, N).transpose(0, 3, 1, 2, 4) \
            .reshape(OH, N, N)
        ht = res.results[core]["ht"].astype(np.float32)         # [128, 4N]
        tailT = np.concatenate([ht[0:GA, 0:N], ht[0:GB, N:2 * N]], axis=0)
        headT = np.concatenate([ht[0:GA, 2 * N:3 * N],
                                ht[0:GB, 3 * N:4 * N]], axis=0)  # [D, N]
        Ap = WhD[osl, 0:D] @ headT                               # [OH, N]
        Bp = WtD[osl, 0:D] @ tailT
        out[b, osl] = (dev + E[osl]
                       + Ap[:, :, None] + Bp[:, None, :])
    return out


# revision 39
# speedup vs baseline: 1.0691x; 1.0691x over previous
"""Trainium2 Bass kernel for nn_CNN_Nested (W2NER-style CNN scorer).

Math (reference):
  head = leaky(wr @ head_w.T + head_b); tail likewise           [B,N,D]
  scores1[b,(h,d),l,k] = sum_{x,y} head[b,l,h,x] U[h,d,x,y] tail[b,k,h,y]
  scores2[b,c,m,n] = h_aug@Wh.T (bcast n) + t_aug@Wt.T (bcast m) + size-emb
  out = down_w @ (scores1+scores2) + down_b                     [B,OUT,N,N]

down_fc is linear => fold down_w into the constants on the host:
  U'[o,h,x,y] = sum_d down_w[o,h*HD+d] U[h,d,x,y]
  WhD = down_w @ Wh, WtD = down_w @ Wt               (tiny)
  E[o,m,n] = (size_emb @ (down_w@Ws).T)[clip(n-m)+15, o] (+ consts)

The device computes ONLY the biaffine part:
  G[o] = blockdiag(U'[o])^T @ tailT                  [(h,x)=200, N]
  dev[o] = headT^T @ G[o]                            [N, N]  (bf16 out)
headT/tailT (the leaky MLP outputs) are returned as a tiny extra
output; the HOST computes the rank-1 broadcasts A' = WhD@headT,
B' = WtD@tailT, the Toeplitz size-embed term E, down_b and the
ones-column constants, all in f32:
  out = dev + A'[:, :, None] + B'[:, None, :] + E.

Hardware notes baked into the schedule (from NTFF traces):
- DMA: ~18ns/descriptor, one per partition row; rows aggregate into
  large bursts ONLY when the DRAM side is a whole transfer-shaped
  tensor, so every transfer gets its own DRAM tensor. gpsimd's
  software DGE fans packets across all 16 DMA engines in parallel and
  carries the bulk; sync/scalar (HWDGE) take the latency-critical
  first chunks.
- PSUM: 8 banks; a static bank plan (A/B group psums sharing banks at
  disjoint columns) avoids pool recycling, whose write-after-read
  hazards would serialize the g stage behind all four MLP evictions.
- PE: consecutive accumulation into one PSUM bank halves issue rate,
  so the out stage interleaves two banks.

Sharding: 8 cores = B(4) x o-half(2x6). No collectives. Full inputs in,
full output out. Hardcoded B=4,N=256,H=768,D=200,NH=5,HD=40,OUT=12.
"""

import os
import numpy as np

B, N, H = 4, 256, 768
D, NH, HD, SZ, OUT = 200, 5, 40, 25, 12
N_POS = 30
OH = OUT // 2          # o's per core
NCORES = 8
GA, GB = 3 * HD, 2 * HD  # 120 / 80: d-rows in partition group A / B
CH = N + 2 * D           # one blob chunk: [wrt_k | tw_k | hw_k] = 656
MLPC = 6 * CH            # 3936
CBDA = MLPC              # bda image cols [120 rows used]
CPW = CBDA + OH * GA
CBDB = CPW + 4 * OH      # bdb image cols [80 rows used]
TOTC = CBDB + OH * GB    # 5160

_cache = {}
LAST_RESULT = None


def _build_module(has_bias: bool):
    import concourse.bacc as bacc
    import concourse.mybir as mybir
    import concourse.tile as tile
    from concourse.bass import ts
    from contextlib import ExitStack

    dt = mybir.dt
    f32 = dt.float32
    bf = dt.bfloat16
    LRELU = mybir.ActivationFunctionType.Lrelu

    nc = bacc.Bacc("TRN2", target_bir_lowering=False, debug=False,
                   enable_asserts=False, enable_partition_id=False)

    c0_d = nc.dram_tensor("c0", [128, CH], bf, kind="ExternalInput").ap()
    c1_d = nc.dram_tensor("c1", [128, CH], bf, kind="ExternalInput").ap()
    c2_d = nc.dram_tensor("c2", [128, CH], bf, kind="ExternalInput").ap()
    c3_d = nc.dram_tensor("c3", [128, CH], bf, kind="ExternalInput").ap()
    c45_d = nc.dram_tensor("c45", [128, 2 * CH], bf,
                           kind="ExternalInput").ap()
    cc_d = nc.dram_tensor("cc", [128, TOTC - MLPC], bf,
                          kind="ExternalInput").ap()
    if has_bias:
        bias_d = nc.dram_tensor("bias", [GA, 4], f32, kind="ExternalInput").ap()
    out_d = nc.dram_tensor("out", [3, 2, 128, 512], bf,
                           kind="ExternalOutput").ap()
    # the very last 512-col half ships as two quarter blocks on two
    # queues so its cast+store pipeline is half as deep
    o2q_d = nc.dram_tensor("o2q", [2, 128, 256], bf,
                           kind="ExternalOutput").ap()
    ht_d = nc.dram_tensor("ht", [128, 4 * N], bf, kind="ExternalOutput").ap()

    with tile.TileContext(nc) as tc, ExitStack() as ctx:
        sb = ctx.enter_context(tc.tile_pool(name="sb", bufs=1))
        # static PSUM plan: 4 MLP banks + 2 g banks + 2 out banks = 8.
        # (Interleaved accumulation chains must NOT share a bank even at
        # disjoint columns -- the accumulate read-modify-write races.)
        pp = ctx.enter_context(tc.tile_pool(name="pp", bufs=1, space="PSUM"))
        pg = ctx.enter_context(tc.tile_pool(name="pg", bufs=2, space="PSUM"))
        po = ctx.enter_context(tc.tile_pool(name="po", bufs=2, space="PSUM"))

        c0_s = sb.tile([128, CH], bf, tag="c0", name="c0")
        nc.sync.dma_start(c0_s[:], c0_d[:, :])
        c1_s = sb.tile([128, CH], bf, tag="c1", name="c1")
        nc.gpsimd.dma_start(c1_s[:], c1_d[:, :])
        c2_s = sb.tile([128, CH], bf, tag="c2", name="c2")
        nc.scalar.dma_start(c2_s[:], c2_d[:, :])
        c3_s = sb.tile([128, CH], bf, tag="c3", name="c3")
        nc.sync.dma_start(c3_s[:], c3_d[:, :])
        c45_s = sb.tile([128, 2 * CH], bf, tag="c45", name="c45")
        nc.gpsimd.dma_start(c45_s[:], c45_d[:, :])
        cc_s = sb.tile([128, TOTC - MLPC], bf, tag="cc", name="cc")
        nc.gpsimd.dma_start(cc_s[:], cc_d[:, :])
        if has_bias:
            bias_s = sb.tile([GA, 4], f32, tag="bias", name="bias")
            nc.scalar.dma_start(bias_s[:], bias_d[:, :])

        def _seg(k):
            if k == 0:
                return c0_s, 0
            if k == 1:
                return c1_s, 0
            if k == 2:
                return c2_s, 0
            if k == 3:
                return c3_s, 0
            return c45_s, (k - 4) * CH

        def wrT(k):
            t, c = _seg(k)
            return t[:, c:c + N]

        def tw_slice(k, off, sz):
            t, c = _seg(k)
            return t[:, c + N + off:c + N + off + sz]

        def hw_slice(k, off, sz):
            t, c = _seg(k)
            return t[:, c + N + D + off:c + N + D + off + sz]

        def bda_sl(j):
            return cc_s[0:GA, j * GA:(j + 1) * GA]

        def bdb_sl(j):
            c0 = CBDB - MLPC
            return cc_s[0:GB, c0 + j * GB:c0 + (j + 1) * GB]

        # ---- headT/tailT = leaky(w @ wr^T + b), [d, l] layout ---------------
        # Chunk-major over the H contraction; all four leaky outputs
        # land in ONE [128,1024] tile so a single aggregated DMA ships
        # them to the host for the A'/B' projections.
        ht_s = sb.tile([128, 4 * N], bf, tag="ht", name="ht")
        tailT_A = ht_s[0:GA, 0 * N:1 * N]
        tailT_B = ht_s[0:GB, 1 * N:2 * N]
        headT_A = ht_s[0:GA, 2 * N:3 * N]
        headT_B = ht_s[0:GB, 3 * N:4 * N]

        pm = {t: pp.tile([sz, N], f32, tag=f"pm{t}", name=f"pm{t}", bufs=1)
              for t, sz in (("tA", GA), ("tB", GB), ("hA", GA), ("hB", GB))}
        groups = [
            ("tA", tw_slice, 0, GA, pm["tA"][:], tailT_A, 2),
            ("tB", tw_slice, GA, GB, pm["tB"][:], tailT_B, 3),
            ("hA", hw_slice, 0, GA, pm["hA"][:], headT_A, 0),
            ("hB", hw_slice, GA, GB, pm["hB"][:], headT_B, 1),
        ]
        for k in range(6):
            for tag, wsl, off, sz, ps, _, _ in groups:
                nc.tensor.matmul(ps, wsl(k, off, sz), wrT(k),
                                 start=(k == 0), stop=(k == 5))

        for tag, _, off, sz, ps, dst, bc in groups:
            if tag == "tB" and not has_bias:
                # run the second tail eviction on the vector engine so
                # the g stage isn't gated on scalar's serial queue
                tmp = sb.tile([GB, N], f32, tag="ltmp", name="ltmp")
                nc.vector.tensor_scalar_mul(tmp[:], ps, 0.01)
                nc.vector.tensor_max(dst, ps, tmp[:])
                continue
            bias = bias_s[0:sz, bc:bc + 1] if has_bias else 0.0
            nc.scalar.activation(dst, ps, LRELU, bias=bias, alpha=0.01)
        nc.sync.dma_start(ht_d[:, :], ht_s[:])

        gAt, gBt = [], []

        def g_build(p):
            gA = sb.tile([GA, 512], bf, tag=f"gA{p}", name=f"gA{p}")
            gB = sb.tile([GB, 512], bf, tag=f"gB{p}", name=f"gB{p}")
            for half in range(2):
                j = 2 * p + half
                # one PSUM bank holds both group psums at disjoint cols
                psg = pg.tile([GA, 512], f32, tag="psg", name="psg")
                nc.tensor.matmul(psg[:, 0:N], bda_sl(j),
                                 tailT_A, start=True, stop=True)
                nc.scalar.copy(gA[:, ts(half, N)], psg[:, 0:N])
                nc.tensor.matmul(psg[0:GB, N:2 * N], bdb_sl(j),
                                 tailT_B, start=True, stop=True)
                nc.vector.tensor_copy(gB[:, ts(half, N)], psg[0:GB, N:2 * N])
            gAt.append(gA)
            gBt.append(gB)

        def out_bank(p):
            out_s = sb.tile([128, 1024], bf, tag=f"os{p}", name=f"os{p}")
            obs = [po.tile([128, 512], f32, tag="ob", name="ob")
                   for _ in range(2)]
            # interleave the two PSUM banks: consecutive accumulation
            # into one bank stalls the PE at half rate. The last bank
            # finishes lt=0 completely first so its cast+store overlap
            # the lt=1 matmuls.
            if p < 2:
                for lt in range(2):
                    nc.tensor.matmul(obs[lt][:], headT_A[:, ts(lt, 128)],
                                     gAt[p][:], start=True, stop=False)
                for lt in range(2):
                    nc.tensor.matmul(obs[lt][:], headT_B[:, ts(lt, 128)],
                                     gBt[p][:], start=False, stop=True)
            else:
                for lt in range(2):
                    nc.tensor.matmul(obs[lt][:], headT_A[:, ts(lt, 128)],
                                     gAt[p][:], start=True, stop=False)
                    nc.tensor.matmul(obs[lt][:], headT_B[:, ts(lt, 128)],
                                     gBt[p][:], start=False, stop=True)
            # each 512-col half casts and stores independently, into its
            # own contiguous DRAM block, as soon as its bank stops
            nc.vector.tensor_copy(out_s[:, 0:512], obs[0][:])
            nc.gpsimd.dma_start(out_d[p, 0], out_s[:, 0:512])
            if p < 2:
                nc.scalar.copy(out_s[:, 512:1024], obs[1][:])
                nc.gpsimd.dma_start(out_d[p, 1], out_s[:, 512:1024])
            else:
                nc.vector.tensor_copy(out_s[:, 512:768], obs[1][:, 0:256])
                nc.sync.dma_start(o2q_d[0], out_s[:, 512:768])
                nc.scalar.copy(out_s[:, 768:1024], obs[1][:, 256:512])
                nc.scalar.dma_start(o2q_d[1], out_s[:, 768:1024])

        g_build(0)
        g_build(1)
        out_bank(0)
        g_build(2)
        out_bank(1)
        out_bank(2)

    nc.compile()
    return nc


def _get_module(has_bias: bool):
    key = ("mod", has_bias)
    if key not in _cache:
        _cache[key] = _build_module(has_bias)
    return _cache[key]


def _host_pack(head_w, head_b, tail_w, tail_b, U_mh, size_emb, W, down_w,
               down_b):
    """Fold down_w into the constants; build bf16 input blobs + host E."""
    from ml_dtypes import bfloat16
    f64 = np.float64
    d1 = D + 1
    Wh, Wt, Ws = W[:, :d1], W[:, d1:2 * d1], W[:, 2 * d1:]
    WhD = (down_w.astype(f64) @ Wh.astype(f64)).astype(np.float32)   # [OUT,D+1]
    WtD = (down_w.astype(f64) @ Wt.astype(f64)).astype(np.float32)
    WsD = (down_w.astype(f64) @ Ws.astype(f64)).astype(np.float32)   # [OUT,SZ]
    ct = (size_emb.astype(f64) @ WsD.T.astype(f64)).astype(np.float32)
    dw_r = down_w.reshape(OUT, NH, HD)
    Up = np.einsum('ohd,hdxy->ohxy', dw_r.astype(f64),
                   U_mh.astype(f64)).astype(np.float32)              # [OUT,NH,HD,HD]

    idx = np.arange(N)
    span = np.clip(idx[None, :] - idx[:, None], -N_POS // 2,
                   N_POS // 2 - 1) + N_POS // 2
    # E folds: size-embed term, down_fc bias, both ones-column constants.
    E = (ct[span].transpose(2, 0, 1)
         + (down_b + WhD[:, D] + WtD[:, D])[:, None, None])          # [OUT,N,N]

    has_bias = bool(np.any(head_b) or np.any(tail_b))

    def pack_w(wmat):  # [D,H] -> [128, 6*200]
        return np.ascontiguousarray(
            wmat.T.reshape(6, 128, D).transpose(1, 0, 2).reshape(128, 6 * D))

    hwp = pack_w(head_w)
    twp = pack_w(tail_w)
    blob0 = np.zeros((128, TOTC), np.float32)
    for k in range(6):
        blob0[:, k * CH + N:k * CH + N + D] = twp[:, k * D:(k + 1) * D]
        blob0[:, k * CH + N + D:(k + 1) * CH] = hwp[:, k * D:(k + 1) * D]

    blobs_oh = []
    bias_m = None
    for oh in range(2):
        osl = slice(oh * OH, (oh + 1) * OH)
        UpS = Up[osl]
        blob = blob0.copy()
        for h in range(3):
            for o in range(OH):
                blob[h * HD:(h + 1) * HD,
                     CBDA + o * GA + h * HD:CBDA + o * GA + (h + 1) * HD] = \
                    UpS[o, h].T
        for h in range(2):
            for o in range(OH):
                blob[h * HD:(h + 1) * HD,
                     CBDB + o * GB + h * HD:CBDB + o * GB + (h + 1) * HD] = \
                    UpS[o, 3 + h].T
        blobs_oh.append(blob.astype(bfloat16))
    if has_bias:
        bias_m = np.zeros((GA, 4), np.float32)
        bias_m[:, 0] = head_b[0:GA]
        bias_m[0:GB, 1] = head_b[GA:D]
        bias_m[:, 2] = tail_b[0:GA]
        bias_m[0:GB, 3] = tail_b[GA:D]
    return blobs_oh, bias_m, WhD, WtD, E, has_bias


def _ensure_axon():
    """If a host-side jax.config pinned the cpu platform (e.g. to run the
    reference), switch back to the axon/neuron backend for the device run."""
    import jax
    try:
        if any(getattr(d, 'platform', '') == 'axon' for d in jax.devices()):
            return
    except Exception:
        pass
    try:
        import jax.extend
        jax.config.update('jax_platforms', 'axon')
        jax.extend.backend.clear_backends()
    except Exception:
        pass


def kernel(word_reps, cls_embeding=None, pieces_index=None, loss_mask=None,
           head_w=None, head_b=None, tail_w=None, tail_b=None, U_mh=None,
           size_emb=None, W=None, down_w=None, down_b=None, **_unused):
    global LAST_RESULT
    from concourse import bass_utils
    from ml_dtypes import bfloat16

    word_reps = np.asarray(word_reps, np.float32)
    args = [np.asarray(a, np.float32) for a in
            (head_w, head_b, tail_w, tail_b, U_mh, size_emb, W, down_w,
             down_b)]
    blobs_oh, bias_m, WhD, WtD, E, has_bias = _host_pack(*args)

    nc = _get_module(has_bias)

    wrt_b = []
    for b in range(B):
        wrt = word_reps[b].T.reshape(6, 128, N).transpose(1, 0, 2) \
            .reshape(128, 6 * N)
        wrt_b.append(wrt.astype(bfloat16))
    in_maps = []
    for core in range(NCORES):
        b, oh = core // 2, core % 2
        blob = blobs_oh[oh].copy()
        for k in range(6):
            blob[:, k * CH:k * CH + N] = wrt_b[b][:, k * N:(k + 1) * N]
        m = dict(c0=np.ascontiguousarray(blob[:, 0:CH]),
                 c1=np.ascontiguousarray(blob[:, CH:2 * CH]),
                 c2=np.ascontiguousarray(blob[:, 2 * CH:3 * CH]),
                 c3=np.ascontiguousarray(blob[:, 3 * CH:4 * CH]),
                 c45=np.ascontiguousarray(blob[:, 4 * CH:6 * CH]),
                 cc=np.ascontiguousarray(blob[:, MLPC:TOTC]))
        if has_bias:
            m['bias'] = bias_m
        in_maps.append(m)

    _ensure_axon()

    trace = bool(os.environ.get("KERNEL_TRACE"))
    res = bass_utils.run_bass_kernel_spmd(nc, in_maps, list(range(NCORES)),
                                          trace=trace)
    LAST_RESULT = res

    out = np.empty((B, OUT, N, N), np.float32)
    for core in range(NCORES):
        b, oh = core // 2, core % 2
        osl = slice(oh * OH, (oh + 1) * OH)
        # out_d layout: [p, t, q, (o2, n)] with o = 2p+o2, m = t*128+q
        raw = res.results[core]["out"].copy()
        o2q = res.results[core]["o2q"]                 # [2,128,256] p2-lt1
        raw[2, 1] = np.concatenate([o2q[0], o2q[1]], axis=1)
        dev = raw.astype(np.float32) \
            .reshape(3, 2, 128, 2, N).transpose(0, 3, 1, 2, 4) \
            .reshape(OH, N, N)
        ht = res.results[core]["ht"].astype(np.float32)         # [128, 4N]
        tailT = np.concatenate([ht[0:GA, 0:N], ht[0:GB, N:2 * N]], axis=0)
        headT = np.concatenate([ht[0:GA, 2 * N:3 * N],
                                ht[0:GB, 3 * N:4 * N]], axis=0)  # [D, N]
        Ap = WhD[osl, 0:D] @ headT                               # [OH, N]
        Bp = WtD[osl, 0:D] @ tailT
        out[b, osl] = (dev + E[osl]
                       + Ap[:, :, None] + Bp[:, None, :])
    return out


# revision 44
# speedup vs baseline: 1.0787x; 1.0090x over previous
"""Trainium2 Bass kernel for nn_CNN_Nested (W2NER-style CNN scorer).

Math (reference):
  head = leaky(wr @ head_w.T + head_b); tail likewise           [B,N,D]
  scores1[b,(h,d),l,k] = sum_{x,y} head[b,l,h,x] U[h,d,x,y] tail[b,k,h,y]
  scores2[b,c,m,n] = h_aug@Wh.T (bcast n) + t_aug@Wt.T (bcast m) + size-emb
  out = down_w @ (scores1+scores2) + down_b                     [B,OUT,N,N]

down_fc is linear => fold down_w into the constants on the host:
  U'[o,h,x,y] = sum_d down_w[o,h*HD+d] U[h,d,x,y]
  WhD = down_w @ Wh, WtD = down_w @ Wt               (tiny)
  E[o,m,n] = (size_emb @ (down_w@Ws).T)[clip(n-m)+15, o] (+ consts)

The device computes ONLY the biaffine part:
  G[o] = blockdiag(U'[o])^T @ tailT                  [(h,x)=200, N]
  dev[o] = headT^T @ G[o]                            [N, N]  (bf16 out)
headT/tailT (the leaky MLP outputs) are returned as a tiny extra
output; the HOST computes the rank-1 broadcasts A' = WhD@headT,
B' = WtD@tailT, the Toeplitz size-embed term E, down_b and the
ones-column constants, all in f32:
  out = dev + A'[:, :, None] + B'[:, None, :] + E.

Hardware notes baked into the schedule (from NTFF traces):
- DMA: ~18ns/descriptor, one per partition row; rows aggregate into
  large bursts ONLY when the DRAM side is a whole transfer-shaped
  tensor, so every transfer gets its own DRAM tensor. gpsimd's
  software DGE fans packets across all 16 DMA engines in parallel and
  carries the bulk; sync/scalar (HWDGE) take the latency-critical
  first chunks.
- PSUM: 8 banks; a static bank plan (A/B group psums sharing banks at
  disjoint columns) avoids pool recycling, whose write-after-read
  hazards would serialize the g stage behind all four MLP evictions.
- PE: consecutive accumulation into one PSUM bank halves issue rate,
  so the out stage interleaves two banks.

Sharding: 8 cores = B(4) x o-half(2x6). No collectives. Full inputs in,
full output out. Hardcoded B=4,N=256,H=768,D=200,NH=5,HD=40,OUT=12.
"""

import os
import numpy as np

B, N, H = 4, 256, 768
D, NH, HD, SZ, OUT = 200, 5, 40, 25, 12
N_POS = 30
OH = OUT // 2          # o's per core
NCORES = 8
GA, GB = 3 * HD, 2 * HD  # 120 / 80: d-rows in partition group A / B
CH = N + 2 * D           # one blob chunk: [wrt_k | tw_k | hw_k] = 656
MLPC = 6 * CH            # 3936
CBDA = MLPC              # bda image cols [120 rows used]
CPW = CBDA + OH * GA
CBDB = CPW + 4 * OH      # bdb image cols [80 rows used]
TOTC = CBDB + OH * GB    # 5160

_cache = {}
LAST_RESULT = None


def _build_module(has_bias: bool):
    import concourse.bacc as bacc
    import concourse.mybir as mybir
    import concourse.tile as tile
    from concourse.bass import ts
    from contextlib import ExitStack

    dt = mybir.dt
    f32 = dt.float32
    bf = dt.bfloat16
    LRELU = mybir.ActivationFunctionType.Lrelu

    nc = bacc.Bacc("TRN2", target_bir_lowering=False, debug=False,
                   enable_asserts=False, enable_partition_id=False)

    c0_d = nc.dram_tensor("c0", [128, CH], bf, kind="ExternalInput").ap()
    c1_d = nc.dram_tensor("c1", [128, CH], bf, kind="ExternalInput").ap()
    c2_d = nc.dram_tensor("c2", [128, CH], bf, kind="ExternalInput").ap()
    c3_d = nc.dram_tensor("c3", [128, CH], bf, kind="ExternalInput").ap()
    c45_d = nc.dram_tensor("c45", [128, 2 * CH], bf,
                           kind="ExternalInput").ap()
    cc_d = nc.dram_tensor("cc", [128, TOTC - MLPC], bf,
                          kind="ExternalInput").ap()
    if has_bias:
        bias_d = nc.dram_tensor("bias", [GA, 4], f32, kind="ExternalInput").ap()
    out_d = nc.dram_tensor("out", [3, 2, 128, 512], bf,
                           kind="ExternalOutput").ap()
    # the very last 512-col half ships as two quarter blocks on two
    # queues so its cast+store pipeline is half as deep
    o2q_d = nc.dram_tensor("o2q", [2, 128, 256], bf,
                           kind="ExternalOutput").ap()
    ht_d = nc.dram_tensor("ht", [128, 4 * N], bf, kind="ExternalOutput").ap()

    with tile.TileContext(nc) as tc, ExitStack() as ctx:
        sb = ctx.enter_context(tc.tile_pool(name="sb", bufs=1))
        # static PSUM plan: 4 MLP banks + 2 g banks + 2 out banks = 8.
        # (Interleaved accumulation chains must NOT share a bank even at
        # disjoint columns -- the accumulate read-modify-write races.)
        pp = ctx.enter_context(tc.tile_pool(name="pp", bufs=1, space="PSUM"))
        pg = ctx.enter_context(tc.tile_pool(name="pg", bufs=2, space="PSUM"))
        po = ctx.enter_context(tc.tile_pool(name="po", bufs=2, space="PSUM"))

        c0_s = sb.tile([128, CH], bf, tag="c0", name="c0")
        nc.sync.dma_start(c0_s[:], c0_d[:, :])
        c1_s = sb.tile([128, CH], bf, tag="c1", name="c1")
        nc.gpsimd.dma_start(c1_s[:], c1_d[:, :])
        c2_s = sb.tile([128, CH], bf, tag="c2", name="c2")
        nc.scalar.dma_start(c2_s[:], c2_d[:, :])
        c3_s = sb.tile([128, CH], bf, tag="c3", name="c3")
        nc.sync.dma_start(c3_s[:], c3_d[:, :])
        c45_s = sb.tile([128, 2 * CH], bf, tag="c45", name="c45")
        nc.gpsimd.dma_start(c45_s[:], c45_d[:, :])
        cc_s = sb.tile([128, TOTC - MLPC], bf, tag="cc", name="cc")
        nc.gpsimd.dma_start(cc_s[:], cc_d[:, :])
        if has_bias:
            bias_s = sb.tile([GA, 4], f32, tag="bias", name="bias")
            nc.scalar.dma_start(bias_s[:], bias_d[:, :])

        def _seg(k):
            if k == 0:
                return c0_s, 0
            if k == 1:
                return c1_s, 0
            if k == 2:
                return c2_s, 0
            if k == 3:
                return c3_s, 0
            return c45_s, (k - 4) * CH

        def wrT(k):
            t, c = _seg(k)
            return t[:, c:c + N]

        def tw_slice(k, off, sz):
            t, c = _seg(k)
            return t[:, c + N + off:c + N + off + sz]

        def hw_slice(k, off, sz):
            t, c = _seg(k)
            return t[:, c + N + D + off:c + N + D + off + sz]

        def bda_sl(j):
            return cc_s[0:GA, j * GA:(j + 1) * GA]

        def bdb_sl(j):
            c0 = CBDB - MLPC
            return cc_s[0:GB, c0 + j * GB:c0 + (j + 1) * GB]

        # ---- headT/tailT = leaky(w @ wr^T + b), [d, l] layout ---------------
        # Chunk-major over the H contraction; all four leaky outputs
        # land in ONE [128,1024] tile so a single aggregated DMA ships
        # them to the host for the A'/B' projections.
        ht_s = sb.tile([128, 4 * N], bf, tag="ht", name="ht")
        tailT_A = ht_s[0:GA, 0 * N:1 * N]
        tailT_B = ht_s[0:GB, 1 * N:2 * N]
        headT_A = ht_s[0:GA, 2 * N:3 * N]
        headT_B = ht_s[0:GB, 3 * N:4 * N]

        pm = {t: pp.tile([sz, N], f32, tag=f"pm{t}", name=f"pm{t}", bufs=1)
              for t, sz in (("tA", GA), ("tB", GB), ("hA", GA), ("hB", GB))}
        groups = [
            ("tA", tw_slice, 0, GA, pm["tA"][:], tailT_A, 2),
            ("tB", tw_slice, GA, GB, pm["tB"][:], tailT_B, 3),
            ("hA", hw_slice, 0, GA, pm["hA"][:], headT_A, 0),
            ("hB", hw_slice, GA, GB, pm["hB"][:], headT_B, 1),
        ]
        for k in range(6):
            for tag, wsl, off, sz, ps, _, _ in groups:
                nc.tensor.matmul(ps, wsl(k, off, sz), wrT(k),
                                 start=(k == 0), stop=(k == 5))

        for tag, _, off, sz, ps, dst, bc in groups:
            if tag == "tB" and not has_bias:
                # run the second tail eviction on the vector engine so
                # the g stage isn't gated on scalar's serial queue
                tmp = sb.tile([GB, N], f32, tag="ltmp", name="ltmp")
                nc.vector.tensor_scalar_mul(tmp[:], ps, 0.01)
                nc.vector.tensor_max(dst, ps, tmp[:])
                continue
            bias = bias_s[0:sz, bc:bc + 1] if has_bias else 0.0
            nc.scalar.activation(dst, ps, LRELU, bias=bias, alpha=0.01)
        nc.sync.dma_start(ht_d[:, :], ht_s[:])

        gAt, gBt = [], []

        def g_build(p):
            gA = sb.tile([GA, 512], bf, tag=f"gA{p}", name=f"gA{p}")
            gB = sb.tile([GB, 512], bf, tag=f"gB{p}", name=f"gB{p}")
            for half in range(2):
                j = 2 * p + half
                # one PSUM bank holds both group psums at disjoint cols
                psg = pg.tile([GA, 512], f32, tag="psg", name="psg")
                nc.tensor.matmul(psg[:, 0:N], bda_sl(j),
                                 tailT_A, start=True, stop=True)
                nc.scalar.copy(gA[:, ts(half, N)], psg[:, 0:N])
                nc.tensor.matmul(psg[0:GB, N:2 * N], bdb_sl(j),
                                 tailT_B, start=True, stop=True)
                nc.vector.tensor_copy(gB[:, ts(half, N)], psg[0:GB, N:2 * N])
            gAt.append(gA)
            gBt.append(gB)

        def out_bank(p):
            out_s = sb.tile([128, 1024], bf, tag=f"os{p}", name=f"os{p}")
            obs = [po.tile([128, 512], f32, tag="ob", name="ob")
                   for _ in range(2)]
            # interleave the two PSUM banks: consecutive accumulation
            # into one bank stalls the PE at half rate. The last bank
            # finishes lt=0 completely first so its cast+store overlap
            # the lt=1 matmuls.
            if p < 2:
                for lt in range(2):
                    nc.tensor.matmul(obs[lt][:], headT_A[:, ts(lt, 128)],
                                     gAt[p][:], start=True, stop=False)
                for lt in range(2):
                    nc.tensor.matmul(obs[lt][:], headT_B[:, ts(lt, 128)],
                                     gBt[p][:], start=False, stop=True)
            else:
                for lt in range(2):
                    nc.tensor.matmul(obs[lt][:], headT_A[:, ts(lt, 128)],
                                     gAt[p][:], start=True, stop=False)
                    nc.tensor.matmul(obs[lt][:], headT_B[:, ts(lt, 128)],
                                     gBt[p][:], start=False, stop=True)
            # each 512-col half casts and stores independently, into its
            # own contiguous DRAM block, as soon as its bank stops
            nc.vector.tensor_copy(out_s[:, 0:512], obs[0][:])
            nc.gpsimd.dma_start(out_d[p, 0], out_s[:, 0:512])
            if p < 2:
                nc.scalar.copy(out_s[:, 512:1024], obs[1][:])
                nc.gpsimd.dma_start(out_d[p, 1], out_s[:, 512:1024])
            else:
                nc.vector.tensor_copy(out_s[:, 512:768], obs[1][:, 0:256])
                nc.sync.dma_start(o2q_d[0], out_s[:, 512:768])
                nc.scalar.copy(out_s[:, 768:1024], obs[1][:, 256:512])
                nc.scalar.dma_start(o2q_d[1], out_s[:, 768:1024])

        g_build(0)
        g_build(1)
        out_bank(0)
        g_build(2)
        out_bank(1)
        out_bank(2)

    nc.compile()
    return nc


def _get_module(has_bias: bool):
    key = ("mod", has_bias)
    if key not in _cache:
        _cache[key] = _build_module(has_bias)
    return _cache[key]


def _host_pack(head_w, head_b, tail_w, tail_b, U_mh, size_emb, W, down_w,
               down_b):
    """Fold down_w into the constants; build bf16 input blobs + host E."""
    from ml_dtypes import bfloat16
    f64 = np.float64
    d1 = D + 1
    Wh, Wt, Ws = W[:, :d1], W[:, d1:2 * d1], W[:, 2 * d1:]
    WhD = (down_w.astype(f64) @ Wh.astype(f64)).astype(np.float32)   # [OUT,D+1]
    WtD = (down_w.astype(f64) @ Wt.astype(f64)).astype(np.float32)
    WsD = (down_w.astype(f64) @ Ws.astype(f64)).astype(np.float32)   # [OUT,SZ]
    ct = (size_emb.astype(f64) @ WsD.T.astype(f64)).astype(np.float32)
    dw_r = down_w.reshape(OUT, NH, HD)
    Up = np.einsum('ohd,hdxy->ohxy', dw_r.astype(f64),
                   U_mh.astype(f64)).astype(np.float32)              # [OUT,NH,HD,HD]

    idx = np.arange(N)
    span = np.clip(idx[None, :] - idx[:, None], -N_POS // 2,
                   N_POS // 2 - 1) + N_POS // 2
    # E folds: size-embed term, down_fc bias, both ones-column constants.
    E = (ct[span].transpose(2, 0, 1)
         + (down_b + WhD[:, D] + WtD[:, D])[:, None, None])          # [OUT,N,N]

    has_bias = bool(np.any(head_b) or np.any(tail_b))

    def pack_w(wmat):  # [D,H] -> [128, 6*200]
        return np.ascontiguousarray(
            wmat.T.reshape(6, 128, D).transpose(1, 0, 2).reshape(128, 6 * D))

    hwp = pack_w(head_w)
    twp = pack_w(tail_w)
    blob0 = np.zeros((128, TOTC), np.float32)
    for k in range(6):
        blob0[:, k * CH + N:k * CH + N + D] = twp[:, k * D:(k + 1) * D]
        blob0[:, k * CH + N + D:(k + 1) * CH] = hwp[:, k * D:(k + 1) * D]

    blobs_oh = []
    bias_m = None
    for oh in range(2):
        osl = slice(oh * OH, (oh + 1) * OH)
        UpS = Up[osl]
        blob = blob0.copy()
        for h in range(3):
            for o in range(OH):
                blob[h * HD:(h + 1) * HD,
                     CBDA + o * GA + h * HD:CBDA + o * GA + (h + 1) * HD] = \
                    UpS[o, h].T
        for h in range(2):
            for o in range(OH):
                blob[h * HD:(h + 1) * HD,
                     CBDB + o * GB + h * HD:CBDB + o * GB + (h + 1) * HD] = \
                    UpS[o, 3 + h].T
        blobs_oh.append(blob.astype(bfloat16))
    if has_bias:
        bias_m = np.zeros((GA, 4), np.float32)
        bias_m[:, 0] = head_b[0:GA]
        bias_m[0:GB, 1] = head_b[GA:D]
        bias_m[:, 2] = tail_b[0:GA]
        bias_m[0:GB, 3] = tail_b[GA:D]
    return blobs_oh, bias_m, WhD, WtD, E, has_bias


def _ensure_axon():
    """If a host-side jax.config pinned the cpu platform (e.g. to run the
    reference), switch back to the axon/neuron backend for the device run."""
    import jax
    try:
        if any(getattr(d, 'platform', '') == 'axon' for d in jax.devices()):
            return
    except Exception:
        pass
    try:
        import jax.extend
        jax.config.update('jax_platforms', 'axon')
        jax.extend.backend.clear_backends()
    except Exception:
        pass


def kernel(word_reps, cls_embeding=None, pieces_index=None, loss_mask=None,
           head_w=None, head_b=None, tail_w=None, tail_b=None, U_mh=None,
           size_emb=None, W=None, down_w=None, down_b=None, **_unused):
    global LAST_RESULT
    from concourse import bass_utils
    from ml_dtypes import bfloat16

    word_reps = np.asarray(word_reps, np.float32)
    args = [np.asarray(a, np.float32) for a in
            (head_w, head_b, tail_w, tail_b, U_mh, size_emb, W, down_w,
             down_b)]
    blobs_oh, bias_m, WhD, WtD, E, has_bias = _host_pack(*args)

    nc = _get_module(has_bias)

    wrt_b = []
    for b in range(B):
        wrt = word_reps[b].T.reshape(6, 128, N).transpose(1, 0, 2) \
            .reshape(128, 6 * N)
        wrt_b.append(wrt.astype(bfloat16))
    in_maps = []
    for core in range(NCORES):
        b, oh = core // 2, core % 2
        blob = blobs_oh[oh].copy()
        for k in range(6):
            blob[:, k * CH:k * CH + N] = wrt_b[b][:, k * N:(k + 1) * N]
        m = dict(c0=np.ascontiguousarray(blob[:, 0:CH]),
                 c1=np.ascontiguousarray(blob[:, CH:2 * CH]),
                 c2=np.ascontiguousarray(blob[:, 2 * CH:3 * CH]),
                 c3=np.ascontiguousarray(blob[:, 3 * CH:4 * CH]),
                 c45=np.ascontiguousarray(blob[:, 4 * CH:6 * CH]),
                 cc=np.ascontiguousarray(blob[:, MLPC:TOTC]))
        if has_bias:
            m['bias'] = bias_m
        in_maps.append(m)

    _ensure_axon()

    trace = bool(os.environ.get("KERNEL_TRACE"))
    res = bass_utils.run_bass_kernel_spmd(nc, in_maps, list(range(NCORES)),
                                          trace=trace)
    LAST_RESULT = res

    out = np.empty((B, OUT, N, N), np.float32)
    for core in range(NCORES):
        b, oh = core // 2, core % 2
        osl = slice(oh * OH, (oh + 1) * OH)
        # out_d layout: [p, t, q, (o2, n)] with o = 2p+o2, m = t*128+q
        raw = res.results[core]["out"].copy()
        o2q = res.results[core]["o2q"]                 # [2,128,256] p2-lt1
        raw[2, 1] = np.concatenate([o2q[0], o2q[1]], axis=1)
        dev = raw.astype(np.float32) \
            .reshape(3, 2, 128, 2, N).transpose(0, 3, 1, 2, 4) \
            .reshape(OH, N, N)
        ht = res.results[core]["ht"].astype(np.float32)         # [128, 4N]
        tailT = np.concatenate([ht[0:GA, 0:N], ht[0:GB, N:2 * N]], axis=0)
        headT = np.concatenate([ht[0:GA, 2 * N:3 * N],
                                ht[0:GB, 3 * N:4 * N]], axis=0)  # [D, N]
        Ap = WhD[osl, 0:D] @ headT                               # [OH, N]
        Bp = WtD[osl, 0:D] @ tailT
        out[b, osl] = (dev + E[osl]
                       + Ap[:, :, None] + Bp[:, None, :])
    return out
